# revision 1
# baseline (speedup 1.0000x reference)
"""Trainium2 Bass kernel for nn_Mlp_13099650253522 (BitNet-ternary dense MLP).

  h = gelu(x @ ter_quant(w1).T + b1);  y = h @ ter_quant(w2).T + b2
  ter_quant(w) = clip(round(w / g), -1, 1) * g,  g = mean(|w|) + 1e-5

v2.3 (8 NeuronCores, data-parallel over the 64*197=12608 tokens):
 - w1 as 6 column-slice tiles [128, kd=6, 512] fp32, ALL-parallel
   DMAs (staggered completions); DVE XY-reduces chase the landings;
   gamma1 via gpsimd partition_all_reduce (pre-warmed).
 - All ternary quant on DVE, {-1,0,1}, 2 passes per job:
   b = (w <= -g/2);  t = (w >= g/2) - b  (scalar_tensor_tensor).
   w1 jobs per (slice, kd) [128,512] slice-major so fc1's hc-outer
   loop chases quant; w2 jobs per (batch, dc-half).
 - x supertiles bf16 after w1; w2 fp32 row-batches all-parallel after
   x (single pass, reusing the w1 pool slots freed by quant).
 - gamma2: ACT Abs+accum_out per w2 batch, interleaved into the fc1
   gelu emission stream (1 per 3 gelus from hc>=8) so the in-order
   ACT queue never starves the PE's 4-deep PSUM ring.
 - fc1 hc-outer: 12 bf16 matmuls per (hc, super) into 2-bank PSUM
   tiles; one ACT Gelu (scale=gamma1, bias=b1) -> bf16 h.
 - fc2 kh-outer in two dc-groups of 3: spreads t2-quant consumption
   over the whole group so DVE keeps ahead; ACT Identity epilogue
   (psum*gamma2+b2) -> bf16 y; DMA out on the gpsimd queue; host
   upcasts to fp32.
"""
import sys

for _p in ("/root/.axon_site", "/root/.axon_site/_ro/trn_rl_repo",
           "/root/.axon_site/_ro/pypackages", "/opt/trn_rl_repo"):
    if _p not in sys.path:
        sys.path.append(_p)

import ml_dtypes
import numpy as np

from concourse import bacc
import concourse.mybir as mybir
from concourse import bass_isa
from concourse.tile import TileContext
from concourse.tile_rust import add_dep_helper
from concourse.bass_utils import run_bass_kernel_spmd

FP32 = mybir.dt.float32
BF16 = mybir.dt.bfloat16
FP8 = mybir.dt.float8e4
Act = mybir.ActivationFunctionType
Alu = mybir.AluOpType
AxX = mybir.AxisListType.X
AxXY = mybir.AxisListType.XY

N_CORES = 8
B, S, D, H = 64, 197, 768, 3072
TOK = B * S                 # 12608
TOK_PER = TOK // N_CORES    # 1576
NS = 2                      # token supertiles per core
STN = TOK_PER // NS         # 788
TN = STN // 2               # 394 (psum bank region)
KD = D // 128               # 6
KH = H // 128               # 24
NSL = 6                     # w1 column-slice count (hc quads)
W1S = H // NSL              # 512
EPS = 1e-5


def build():
    nc = bacc.Bacc("TRN2", target_bir_lowering=False, debug=False)
    xt = nc.declare_dram_parameter("xt", [128, NS, KD, STN], BF16,
                                   isOutput=False)
    wt1 = nc.declare_dram_parameter("wt1", [D, H], FP32, isOutput=False)
    wt2 = nc.declare_dram_parameter("wt2", [H, D], FP32, isOutput=False)
    b1r = nc.declare_dram_parameter("b1r", [128, KH], FP32, isOutput=False)
    b2r = nc.declare_dram_parameter("b2r", [128, KD], FP32, isOutput=False)
    yt = nc.declare_dram_parameter("yt", [D, TOK_PER], BF16, isOutput=True)

    with TileContext(nc) as tc:
        with (
            tc.tile_pool(name="singles", bufs=1) as singles,
            tc.tile_pool(name="wbig", bufs=6) as wbig,   # w1/w2 fp32 72K
            tc.tile_pool(name="t1", bufs=36) as t1p,     # ternary w1 18K
            tc.tile_pool(name="t2", bufs=12) as t2p,     # ternary w2 18K
            tc.tile_pool(name="xp", bufs=1) as xp,       # x bf16 19K
            tc.tile_pool(name="hp", bufs=24) as hp,      # h bf16 38K
            tc.tile_pool(name="yp", bufs=3) as yp,       # y staging 4.7K
            tc.tile_pool(name="qp", bufs=2) as qp,       # quant transients
            tc.tile_pool(name="dp", bufs=1) as dp,       # ACT abs dump
            tc.tile_pool(name="ps", bufs=4, space="PSUM") as psp,
        ):
            # warm the gpsimd custom-op library early
            dmy = singles.tile([128, 2], FP32, tag="dmy")
            nc.gpsimd.memset(dmy, 0.0)
            dmy2 = singles.tile([128, 1], FP32, tag="dmy2")
            nc.gpsimd.partition_all_reduce(dmy2, dmy[:, 0:1], channels=128,
                                           reduce_op=bass_isa.ReduceOp.add)

            def gamma_chain(acc_cols, total_elems, tag):
                rsum = singles.tile([128, 1], FP32, tag=tag + "_rs")
                nc.vector.tensor_reduce(out=rsum[:, 0:1], in_=acc_cols,
                                        axis=AxX, op=Alu.add)
                allr = singles.tile([128, 1], FP32, tag=tag + "_ar")
                nc.gpsimd.partition_all_reduce(allr, rsum, channels=128,
                                               reduce_op=bass_isa.ReduceOp.add)
                gf = singles.tile([128, 1], FP32, tag=tag + "_gf")
                nc.vector.tensor_scalar(
                    out=gf, in0=allr, scalar1=1.0 / total_elems,
                    scalar2=EPS, op0=Alu.mult, op1=Alu.add)
                gh = singles.tile([128, 1], FP32, tag=tag + "_gh")
                nc.vector.tensor_scalar_mul(gh, gf, 0.5)
                gn = singles.tile([128, 1], FP32, tag=tag + "_gn")
                nc.vector.tensor_scalar_mul(gn, gf, -0.5)
                return gf, gh, gn

            def quant_dve(w_in, dst, gh, gn, btag, bbufs=2):
                """2-pass ternary -> {-1,0,1}: b=(w<=-g/2); t=(w>=g/2)-b"""
                b = qp.tile(list(w_in.shape), FP32, tag=btag, bufs=bbufs)
                nc.vector.tensor_scalar(out=b, in0=w_in, scalar1=gn[:, 0:1],
                                        scalar2=0.0, op0=Alu.is_le,
                                        op1=Alu.add)
                nc.vector.scalar_tensor_tensor(out=dst, in0=w_in,
                                               scalar=gh[:, 0:1], in1=b,
                                               op0=Alu.is_ge,
                                               op1=Alu.subtract)

            # ---- w1: 6 column-slice tiles, all-parallel DMAs ----
            acc1 = singles.tile([128, NSL], FP32, tag="acc1")
            w1t = []
            w1_dmas = []
            for s in range(NSL):
                wf = wbig.tile([128, KD, W1S], FP32, tag="wbig")
                src = wt1[:, s * W1S:(s + 1) * W1S]
                dma = nc.sync.dma_start(
                    out=wf, in_=src.rearrange("(k p) c -> p k c", p=128))
                w1_dmas.append(dma)
                w1t.append(wf)
                nc.vector.tensor_reduce(out=acc1[:, s:s + 1], in_=wf,
                                        axis=AxXY, op=Alu.add,
                                        apply_absolute_value=True)
            g1f, g1h, g1n = gamma_chain(acc1, D * H, "g1")

            b1sb = singles.tile([128, KH], FP32, tag="b1sb")
            d_b1 = nc.sync.dma_start(out=b1sb, in_=b1r[:, :])
            add_dep_helper(d_b1.ins, w1_dmas[0].ins, reason="b1 after w1[0]")
            b2sb = singles.tile([128, KD], FP32, tag="b2sb")
            d_b2 = nc.sync.dma_start(out=b2sb, in_=b2r[:, :])
            add_dep_helper(d_b2.ins, w1_dmas[0].ins, reason="b2 after w1[0]")

            # ---- x supertiles (x0 after mid-w1, x1 after last w1) ----
            xs = []
            x_dmas = []
            for s in range(NS):
                xst = xp.tile([128, KD, STN], BF16, tag=f"xs{s}")
                dma = nc.sync.dma_start(out=xst, in_=xt[:, s, :, :])
                gate = w1_dmas[2] if s == 0 else w1_dmas[-1]
                add_dep_helper(dma.ins, gate.ins, reason="x gating")
                x_dmas.append(dma)
                xs.append(xst)

            # ---- w1 quant: slice-major (s, kd) jobs, all on DVE ----
            t1sk = [[None] * KD for _ in range(NSL)]
            for s in range(NSL):
                for kd in range(KD):
                    t1tile = t1p.tile([128, W1S], FP8, tag="t1")
                    t1sk[s][kd] = t1tile
            for s in range(NSL):
                for kd in range(KD):
                    quant_dve(w1t[s][:, kd, :], t1sk[s][kd],
                              g1h, g1n, "qb1")

            # ---- w2 fp32 row-batches: all-parallel into wbig slots ----
            w2t = []
            w2_dmas = []
            for bt in range(KD):
                wf = wbig.tile([128, 4, D], FP32, tag="wbig")
                src = wt2[bt * 512:(bt + 1) * 512, :]
                dma = nc.sync.dma_start(
                    out=wf, in_=src.rearrange("(c p) f -> p c f", p=128))
                add_dep_helper(dma.ins, x_dmas[-1].ins,
                               reason="w2 after x")
                w2_dmas.append(dma)
                w2t.append(wf)

            # gamma2 accumulators; ABS ops are emitted inside fc1 below
            acc2 = singles.tile([128, KD], FP32, tag="acc2")
            junk2 = dp.tile([128, 4 * D], BF16, tag="junk2")
            w2_abs_queue = list(range(KD))

            # t2 tiles per (batch, dc-half): [128, 4, 384] fp8
            t2bh = [[None, None] for _ in range(KD)]
            for bt in range(KD):
                for hf in range(2):
                    t2tile = t2p.tile([128, 4, 384], FP8, tag="t2")
                    t2bh[bt][hf] = t2tile

            # ---- fc1 / fc2 per supertile ----
            hbt = {}

            def emit_w2_abs(bt):
                nc.scalar.activation(junk2, w2t[bt], Act.Abs,
                                     accum_out=acc2[:, bt:bt + 1])

            def fc1(s, interleave_abs=False):
                for hc in range(KH):
                    sl, off = hc // 4, (hc % 4) * 128
                    ps = psp.tile([128, 2, 512], FP32, tag="ps")
                    for kd in range(KD):
                        lhsT = t1sk[sl][kd][:, off:off + 128]
                        for blk in range(2):
                            nc.tensor.matmul(
                                ps[:, blk, 0:TN], lhsT,
                                xs[s][:, kd, blk * TN:(blk + 1) * TN],
                                start=(kd == 0), stop=(kd == KD - 1))
                    hbv = hp.tile([128, 2, TN], BF16, tag="hb")
                    nc.scalar.activation(hbv, ps[:, :, 0:TN], Act.Gelu,
                                         bias=b1sb[:, hc:hc + 1],
                                         scale=g1f[:, 0:1])
                    hbt.setdefault(s, []).append(hbv)
                    if interleave_abs and hc >= 8 and hc % 3 == 2 \
                            and w2_abs_queue:
                        emit_w2_abs(w2_abs_queue.pop(0))

            def fc2(s, g2f_, g2h_):
                for g in range(2):
                    pss = []
                    for dci in range(3):
                        pst = psp.tile([128, 2, 512], FP32, tag="ps")
                        pss.append(pst)
                    for kh in range(KH):
                        for dci in range(3):
                            lhsT = t2bh[kh // 4][g][:, kh % 4,
                                                    dci * 128:dci * 128 + 128]
                            for blk in range(2):
                                nc.tensor.matmul(
                                    pss[dci][:, blk, 0:TN], lhsT,
                                    hbt[s][kh][:, blk, :],
                                    start=(kh == 0), stop=(kh == KH - 1))
                    for dci in range(3):
                        dc = 3 * g + dci
                        ysb = yp.tile([128, 2, TN], BF16, tag="ysb")
                        nc.scalar.activation(ysb, pss[dci][:, :, 0:TN],
                                             Act.Identity,
                                             bias=b2sb[:, dc:dc + 1],
                                             scale=g2f_[:, 0:1])
                        eng = nc.gpsimd if dci % 2 == 0 else nc.sync
                        eng.dma_start(
                            out=yt[dc * 128:(dc + 1) * 128,
                                   s * STN:(s + 1) * STN],
                            in_=ysb)
                del hbt[s]

            fc1(0, interleave_abs=True)
            while w2_abs_queue:
                emit_w2_abs(w2_abs_queue.pop(0))
            g2f, g2h, g2n = gamma_chain(acc2, D * H, "g2")
            # w2 quant: dc-half-0 of all batches first (fc2 group 0
            # consumes them kh-outer), then dc-half-1.
            for hf in range(2):
                for bt in range(KD):
                    quant_dve(w2t[bt][:, :, hf * 384:(hf + 1) * 384],
                              t2bh[bt][hf], g2h, g2n, "qb2")
            fc2(0, g2f, g2h)
            fc1(1)
            fc2(1, g2f, g2h)

    nc.compile()
    return nc


_NC = None


def _get_nc():
    global _NC
    if _NC is None:
        _NC = build()
    return _NC


def kernel(x, w1, b1, w2, b2, _trace=False, _trace_kwargs=None):
    nc = _get_nc()
    x = np.asarray(x, dtype=np.float32)
    w1 = np.asarray(w1, dtype=np.float32)
    b1 = np.asarray(b1, dtype=np.float32)
    w2 = np.asarray(w2, dtype=np.float32)
    b2 = np.asarray(b2, dtype=np.float32)
    x2 = np.ascontiguousarray(x.reshape(TOK, D).T).astype(ml_dtypes.bfloat16)
    wt1 = np.ascontiguousarray(w1.T)                    # [768, 3072]
    wt2 = np.ascontiguousarray(w2.T)                    # [3072, 768]
    b1r = np.ascontiguousarray(b1.reshape(KH, 128).T)   # [128, 24]
    b2r = np.ascontiguousarray(b2.reshape(KD, 128).T)   # [128, 6]
    in_maps = []
    for c in range(N_CORES):
        xc = x2[:, c * TOK_PER:(c + 1) * TOK_PER]       # [768, 1576]
        xc = xc.reshape(KD, 128, NS, STN).transpose(1, 2, 0, 3)
        in_maps.append({
            "xt": np.ascontiguousarray(xc),
            "wt1": wt1, "wt2": wt2, "b1r": b1r, "b2r": b2r,
        })
    out = run_bass_kernel_spmd(nc, in_maps, list(range(N_CORES)),
                               trace=_trace, **(_trace_kwargs or {}))
    res = out.results
    yt = np.concatenate([res[c]["yt"].astype(np.float32) for c in
                         range(N_CORES)], axis=1)
    y = np.ascontiguousarray(yt.T).reshape(B, S, D)
    if _trace:
        return y, out
    return y

